# revision 1
# baseline (speedup 1.0000x reference)
"""Trainium2 Bass kernel for a 2-layer GAT (GNN message passing).

Strategy (8 NeuronCores, SPMD, single launch):
  - Destination-shard nodes: core c owns dst nodes [c*12500, (c+1)*12500).
    Each core receives all edges into its nodes -> segment softmax needs no
    cross-core reduction.
  - Node phase 1 on each core: h1 = x_slice @ [W1 | W1@A_s | W1@A_d] on PE,
    rows [h1(64) | alpha_s(8) | alpha_d(8)] stored as 256B bf16 rows.
  - Table AllGather is split into 4 quarter collectives (one per gather
    bank) so edge processing of bank b starts as soon as quarter b lands.
  - Edge phase (bank-major): dma_gather fetches per-edge src rows in
    4096-index batches (int16 indices relative to one of 4 banks of
    25088 rows).  Per 128-edge chunk, segment aggregation is a PE matmul
    with a selection matrix built from an iota compare; alpha_dst is
    expanded per-edge with the transposed selection matrix (built by a
    DVE compare directly against a PE-broadcast PSUM row); softmax
    denominators ride along as extra matmul columns; the division is
    deferred to a per-node post-scale.
  - Self-loop edges are *not* gathered: their contribution (w=exp(lrelu(
    alpha_s+alpha_d)) to numerator+denominator) is computed node-locally
    per tile, which also initializes the aggregation buffers.
  - Node phase 2: divide, +bias, ELU, @ [W2 | folded attention vectors],
    quarter AllGathers, edge phase 2 (identical edge schedule), epilogue
    (divide, bias, log-softmax).
"""

import sys

sys.path.insert(0, "/opt/trn_rl_repo")

import numpy as np
import ml_dtypes

import concourse.bass as bass
import concourse.bacc as bacc
import concourse.mybir as mybir
from concourse.tile import TileContext
from concourse.bass_utils import run_bass_kernel_spmd

import os

BF16 = ml_dtypes.bfloat16
P = 128
NCORES = 8
# chunks per dma_gather call (x128 = indices per call).  The SWDGE
# descriptor rings overflow (ring-space wait deadlocks) for larger
# calls; 8 chunks (1024 indices, 65 descriptors/ring) is validated.
BATCH_CHUNKS = int(os.environ.get("BATCH_CHUNKS", "8"))
SPLIT_AG = int(os.environ.get("SPLIT_AG", "0"))   # 4 quarter-AGs vs 1 full

# ---------------------------------------------------------------- config


class Cfg:
    def __init__(self, n_nodes, n_edges, f_in, heads1, out1, n_classes,
                 npc, nbank, neg_slope=0.2):
        self.N = n_nodes
        self.E = n_edges
        self.F_IN = f_in                    # 256
        self.H1 = heads1                    # 8
        self.O1 = out1                      # 8
        self.C = n_classes                  # 40
        self.NEG = neg_slope
        self.NPC = npc                      # raw nodes per core
        assert npc * NCORES >= n_nodes
        self.TILES = (npc + P - 1) // P
        self.NPAD = self.TILES * P          # padded nodes per core
        self.NTOT = NCORES * self.NPAD      # table rows
        self.NBANK = nbank
        assert self.NPAD % nbank == 0
        self.QUART = self.NPAD // nbank     # rows per core per bank
        self.BANK = NCORES * self.QUART     # rows per bank
        assert self.BANK <= 32768
        self.D1 = heads1 * out1             # 64
        self.F1 = self.D1 + heads1          # 72 (msg cols + denom cols)
        self.F2 = n_classes + 1             # 41
        self.ROW1 = 128                     # bf16 elems/row in table1 (256B)
        self.ROW2 = 64                      # fp32 elems/row in table2 (256B)
        assert self.D1 + 2 * heads1 <= self.ROW1
        assert n_classes + 2 <= self.ROW2
        self.KCH = (f_in + P - 1) // P      # k-chunks in node matmul 1


FULL = Cfg(n_nodes=100000, n_edges=1600000, f_in=256, heads1=8, out1=8,
           n_classes=40, npc=12500, nbank=4)


# ------------------------------------------------------- host preprocessing


def build_edge_meta(cfg, src, dst):
    """Partition/sort/pad (non-self-loop) edges.

    Table row layout (matches the 4 quarter-AllGathers): node n with
    core c = n // NPC, local l = n % NPC sits in bank q = l // QUART at
    bank-index c * QUART + (l % QUART).

    Returns (meta, per-core idx/drel streams).  meta is identical across
    cores: per bank chunk list [(tile, start, stop)], 32-chunk batches,
    global chunk offsets per bank.
    """
    s_core, s_loc = src // cfg.NPC, src % cfg.NPC
    if SPLIT_AG:
        bank = s_loc // cfg.QUART
        bidx = s_core * cfg.QUART + (s_loc % cfg.QUART)
    else:
        src_row = s_core * cfg.NPAD + s_loc
        bank = src_row // cfg.BANK
        bidx = src_row % cfg.BANK
    dst_core = dst // cfg.NPC
    dst_loc = dst % cfg.NPC
    tile = dst_loc // P
    drel = dst_loc % P

    counts = np.zeros((NCORES, cfg.NBANK, cfg.TILES), np.int64)
    np.add.at(counts, (dst_core, bank, tile), 1)
    K = np.ceil(counts.max(axis=0) / P).astype(np.int64)      # [NBANK, TILES]

    # pad each bank's chunk count to a multiple of 4 (group granularity)
    for b in range(cfg.NBANK):
        tot = int(K[b].sum())
        extra = (-tot) % 4
        if extra and tot > 0:
            tstar = int(np.nonzero(K[b])[0][-1])
            K[b, tstar] += extra

    chunks = []          # per bank: list of (tile, start, stop)
    batches = []         # per bank: list of (lo, hi)
    for b in range(cfg.NBANK):
        ch = []
        for t in range(cfg.TILES):
            k = int(K[b, t])
            for i in range(k):
                ch.append((t, i == 0, i == k - 1))
        chunks.append(ch)
        bt = []
        lo = 0
        while lo < len(ch):
            hi = min(lo + BATCH_CHUNKS, len(ch))
            bt.append((lo, hi))
            lo = hi
        batches.append(bt)

    nch_bank = [len(c) for c in chunks]
    nch_tot = sum(nch_bank)
    bank_off = np.cumsum([0] + nch_bank)[:-1]

    order_key = (dst_core * cfg.NBANK + bank) * cfg.TILES + tile
    perm = np.argsort(order_key, kind="stable")
    s_core_, s_bank, s_tile = dst_core[perm], bank[perm], tile[perm]
    s_bidx, s_drel = bidx[perm], drel[perm]

    gidx_all = np.zeros((NCORES, nch_tot * P), np.int16)
    drel_all = np.full((NCORES, nch_tot * P), -1.0, np.float32)

    run_off = np.zeros((NCORES, cfg.NBANK, cfg.TILES), np.int64)
    for b in range(cfg.NBANK):
        off = 0
        for t in range(cfg.TILES):
            run_off[:, b, t] = bank_off[b] * P + off * P
            off += int(K[b, t])
    grp = s_core_ * (cfg.NBANK * cfg.TILES) + s_bank * cfg.TILES + s_tile
    first = np.r_[True, grp[1:] != grp[:-1]]
    gstart = np.maximum.accumulate(np.where(first, np.arange(len(grp)), 0))
    within = np.arange(len(grp)) - gstart
    pos = run_off[s_core_, s_bank, s_tile] + within
    gidx_all[s_core_, pos] = s_bidx.astype(np.int16)
    drel_all[s_core_, pos] = s_drel.astype(np.float32)

    meta = dict(K=K, chunks=chunks, batches=batches, bank_off=bank_off,
                nch_tot=nch_tot)
    return meta, gidx_all, drel_all


def wrap_idx(gidx_flat):
    """idx stream [E] -> dma_gather layout [128, E/16] (16-lane wrap,
    replicated into the 8 sixteen-partition groups)."""
    e = gidx_flat.shape[0]
    assert e % 16 == 0
    w = gidx_flat.reshape(e // 16, 16).T          # [16, E/16]
    return np.tile(w, (8, 1)).astype(np.int16)     # [128, E/16]


def prep_core_inputs(cfg, meta, core, x, W1, a_s1, a_d1, b1, W2, a_s2, a_d2,
                     b2, gidx_core, drel_core):
    n0, n1 = core * cfg.NPC, min((core + 1) * cfg.NPC, cfg.N)
    xs = np.zeros((cfg.NPAD, cfg.F_IN), np.float32)
    xs[: n1 - n0] = x[n0:n1]
    xT = np.ascontiguousarray(xs.T)                          # [F_IN, NPAD]
    kch = cfg.KCH
    xT_s = np.zeros((kch, P, cfg.NPAD), BF16)
    for k in range(kch):
        lo, hi = k * P, min((k + 1) * P, cfg.F_IN)
        xT_s[k, : hi - lo] = xT[lo:hi].astype(BF16)

    A_s = np.zeros((cfg.D1, cfg.H1), np.float32)
    A_d = np.zeros((cfg.D1, cfg.H1), np.float32)
    for h in range(cfg.H1):
        A_s[h * cfg.O1:(h + 1) * cfg.O1, h] = a_s1[h]
        A_d[h * cfg.O1:(h + 1) * cfg.O1, h] = a_d1[h]
    Wfull = np.concatenate([W1, W1 @ A_s, W1 @ A_d], axis=1)  # [F_IN, 80]
    wall = np.zeros((kch, P, cfg.D1 + 2 * cfg.H1), BF16)
    for k in range(kch):
        lo, hi = k * P, min((k + 1) * P, cfg.F_IN)
        wall[k, : hi - lo] = Wfull[lo:hi].astype(BF16)

    w2aug = np.concatenate(
        [W2, (W2 @ a_s2[0])[:, None], (W2 @ a_d2[0])[:, None]], axis=1
    ).astype(np.float32)

    bias1r = np.tile(b1[None, :], (P, 1)).astype(np.float32)
    bias2r = np.tile(b2[None, :], (P, 1)).astype(np.float32)
    iotar = np.tile(np.arange(P, dtype=np.float32)[None, :], (P, 1)).astype(BF16)
    iotac = np.tile(np.arange(P, dtype=np.float32)[:, None], (1, P)).astype(BF16)
    ones1 = np.ones((1, P), BF16)
    identm = np.eye(P, dtype=np.float32)

    nch = meta["nch_tot"]
    gidx = wrap_idx(gidx_core)                               # [128, nch*8]
    drelc = np.ascontiguousarray(
        drel_core.reshape(nch, P).T).astype(BF16)            # [128, nch]
    rows = []
    for b in range(cfg.NBANK):
        off = meta["bank_off"][b]
        for (lo, hi) in meta["batches"][b]:
            r = np.full((BATCH_CHUNKS * P,), -1.0, np.float32)
            r[: (hi - lo) * P] = drel_core[(off + lo) * P:(off + hi) * P]
            rows.append(r[None, :])
    drelf = (np.stack(rows).astype(BF16) if rows
             else np.zeros((1, 1, BATCH_CHUNKS * P), BF16))

    return dict(xT=xT_s, wall=wall, w2aug=w2aug, bias1r=bias1r, bias2r=bias2r,
                iotar=iotar, iotac=iotac, ones1=ones1, identd=identm,
                gidx=gidx, drelc=drelc, drelf=drelf)


# ------------------------------------------------------------ bass program


def build_program(cfg, meta, phases="ACDFG"):
    nc = bacc.Bacc(None, target_bir_lowering=False, debug=False)
    f32, bf16, i16 = mybir.dt.float32, mybir.dt.bfloat16, mybir.dt.int16

    nch = meta["nch_tot"]
    nbatch_tot = sum(len(b) for b in meta["batches"])

    xT = nc.declare_dram_parameter("xT", [cfg.KCH, P, cfg.NPAD], bf16, isOutput=False)
    wall = nc.declare_dram_parameter("wall", [cfg.KCH, P, cfg.D1 + 2 * cfg.H1], bf16, isOutput=False)
    w2aug = nc.declare_dram_parameter("w2aug", [cfg.D1, cfg.C + 2], f32, isOutput=False)
    bias1r = nc.declare_dram_parameter("bias1r", [P, cfg.D1], f32, isOutput=False)
    bias2r = nc.declare_dram_parameter("bias2r", [P, cfg.C], f32, isOutput=False)
    identd = nc.declare_dram_parameter("identd", [P, P], f32, isOutput=False)
    iotar_d = nc.declare_dram_parameter("iotar", [P, P], bf16, isOutput=False)
    iotac_d = nc.declare_dram_parameter("iotac", [P, P], bf16, isOutput=False)
    ones1_d = nc.declare_dram_parameter("ones1", [1, P], bf16, isOutput=False)
    gidx_d = nc.declare_dram_parameter("gidx", [P, nch * 8], i16, isOutput=False)
    drelc_d = nc.declare_dram_parameter("drelc", [P, nch], bf16, isOutput=False)
    drelf_d = nc.declare_dram_parameter("drelf", [nbatch_tot, 1, BATCH_CHUNKS * P], bf16, isOutput=False)
    out_d = nc.declare_dram_parameter("out", [cfg.NPAD, cfg.C], f32, isOutput=True)

    t1loc = nc.dram_tensor("t1loc", [cfg.NPAD, cfg.ROW1], bf16)
    t2loc = nc.dram_tensor("t2loc", [cfg.NPAD, cfg.ROW2], f32)
    if SPLIT_AG:
        t1bank = [nc.dram_tensor(f"t1bank{b}", [cfg.BANK, cfg.ROW1], bf16,
                                 addr_space="Shared") for b in range(cfg.NBANK)]
        t2bank = [nc.dram_tensor(f"t2bank{b}", [cfg.BANK, cfg.ROW2], f32,
                                 addr_space="Shared") for b in range(cfg.NBANK)]
        t1bank_ap = [h[:] for h in t1bank]
        t2bank_ap = [h[:] for h in t2bank]
    else:
        t1full = nc.dram_tensor("t1full", [cfg.NTOT, cfg.ROW1], bf16,
                                addr_space="Shared")
        t2full = nc.dram_tensor("t2full", [cfg.NTOT, cfg.ROW2], f32,
                                addr_space="Shared")
        t1bank_ap = [t1full[b * cfg.BANK:(b + 1) * cfg.BANK, :]
                     for b in range(cfg.NBANK)]
        t2bank_ap = [t2full[b * cfg.BANK:(b + 1) * cfg.BANK, :]
                     for b in range(cfg.NBANK)]
        t1bank = t2bank = None

    H1, D1, C = cfg.H1, cfg.D1, cfg.C
    F1, F2 = cfg.F1, cfg.F2
    WA = D1 + 2 * H1                                   # 80

    with TileContext(nc) as tc:
        with tc.tile_pool(name="persist", bufs=1) as pp:
            ident = pp.tile([P, P], f32)
            nc.sync.dma_start(out=ident[:], in_=identd[:])
            wall_sb = pp.tile([P, cfg.KCH, WA], bf16)
            for k in range(cfg.KCH):
                nc.sync.dma_start(out=wall_sb[:, k, :], in_=wall[k])
            w2aug_sb = pp.tile([D1, C + 2], f32)
            nc.sync.dma_start(out=w2aug_sb[:], in_=w2aug[:])
            b1_sb = pp.tile([P, D1], f32)
            nc.sync.dma_start(out=b1_sb[:], in_=bias1r[:])
            b2_sb = pp.tile([P, C], f32)
            nc.sync.dma_start(out=b2_sb[:], in_=bias2r[:])
            iotar_sb = pp.tile([P, P], bf16)
            nc.sync.dma_start(out=iotar_sb[:], in_=iotar_d[:])
            iotac_sb = pp.tile([P, P], bf16)
            nc.sync.dma_start(out=iotac_sb[:], in_=iotac_d[:])
            ones1_sb = pp.tile([1, P], bf16)
            nc.sync.dma_start(out=ones1_sb[:], in_=ones1_d[:])
            ad1_buf = pp.tile([P, cfg.TILES * H1], bf16)
            ad2_buf = pp.tile([P, cfg.TILES], bf16)
            sl1_buf = pp.tile([P, cfg.TILES * WA], bf16)
            sl2_buf = pp.tile([P, cfg.TILES * (C + 2)], bf16)
            agg1 = pp.tile([P, cfg.TILES * F1], f32)
            agg2 = pp.tile([P, cfg.TILES * F2], f32)
            o2st = pp.tile([P, cfg.TILES * C], f32)
            sst = pp.tile([P, cfg.TILES], f32)
            lnst = pp.tile([P, cfg.TILES], f32)
            scr1 = pp.tile([cfg.NBANK, cfg.ROW1], bf16)
            scr2 = pp.tile([cfg.NBANK, cfg.ROW2], f32)

            # ---------------- phase A: node transform layer 1 (+ quarter AGs)
            if "A" in phases:
                node_phase1(nc, tc, cfg, xT, wall_sb, ident, ad1_buf,
                            sl1_buf, t1loc, t1bank, scr1)
                if not SPLIT_AG:
                    nc.gpsimd.collective_compute(
                        "AllGather", mybir.AluOpType.bypass,
                        replica_groups=[list(range(NCORES))],
                        ins=[t1loc[:]], outs=[t1full[:]])
                    nc.gpsimd.dma_start(
                        out=scr1[0:1, :],
                        in_=t1full[cfg.NTOT - 1:cfg.NTOT, :])

            # ---------------- phase C: self-loops + edge layer 1
            if "C" in phases:
                self_loops(nc, tc, cfg, layer=1, sl_buf=sl1_buf, agg=agg1)
                edge_phase(nc, tc, cfg, meta, layer=1, banks=t1bank_ap,
                           row_elems=cfg.ROW1, fcols=F1, gdt=bf16,
                           gidx_d=gidx_d, drelc_d=drelc_d, drelf_d=drelf_d,
                           iotar_sb=iotar_sb, iotac_sb=iotac_sb,
                           ones1_sb=ones1_sb, ad_buf=ad1_buf, agg=agg1)

            # ---------------- phase D: node transform layer 2 (+ quarter AGs)
            if "D" in phases:
                node_phase2(nc, tc, cfg, agg1, b1_sb, w2aug_sb, ident,
                            ad2_buf, sl2_buf, t2loc, t2bank, scr2)
                if not SPLIT_AG:
                    nc.gpsimd.collective_compute(
                        "AllGather", mybir.AluOpType.bypass,
                        replica_groups=[list(range(NCORES))],
                        ins=[t2loc[:]], outs=[t2full[:]])
                    nc.gpsimd.dma_start(
                        out=scr2[0:1, :],
                        in_=t2full[cfg.NTOT - 1:cfg.NTOT, :])

            # ---------------- phase F: self-loops + edge layer 2
            if "F" in phases:
                self_loops(nc, tc, cfg, layer=2, sl_buf=sl2_buf, agg=agg2)
                edge_phase(nc, tc, cfg, meta, layer=2, banks=t2bank_ap,
                           row_elems=cfg.ROW2, fcols=F2, gdt=f32,
                           gidx_d=gidx_d, drelc_d=drelc_d, drelf_d=drelf_d,
                           iotar_sb=iotar_sb, iotac_sb=iotac_sb,
                           ones1_sb=ones1_sb, ad_buf=ad2_buf, agg=agg2)

            # ---------------- phase G: epilogue (divide, bias, log_softmax)
            if "G" in phases:
                epilogue(nc, tc, cfg, agg2, b2_sb, o2st, sst, lnst, out_d)

    nc.compile()
    return nc


def quarter_ag(nc, cfg, b, loc, bank_t, scr):
    """AllGather quarter b of the local table into bank b (+ dummy read
    to anchor collective completion for the tile framework)."""
    q = cfg.QUART
    nc.gpsimd.collective_compute(
        "AllGather", mybir.AluOpType.bypass,
        replica_groups=[list(range(NCORES))],
        ins=[loc[b * q:(b + 1) * q, :]], outs=[bank_t[:]])
    nc.gpsimd.dma_start(out=scr[b:b + 1, :],
                        in_=bank_t[cfg.BANK - 1:cfg.BANK, :])


def node_phase1(nc, tc, cfg, xT, wall_sb, ident, ad1_buf, sl1_buf, t1loc,
                t1bank, scr1):
    f32, bf16 = mybir.dt.float32, mybir.dt.bfloat16
    H1, D1 = cfg.H1, cfg.D1
    WA = D1 + 2 * H1
    qt = [(cfg.QUART * (b + 1) + P - 1) // P - 1 for b in range(cfg.NBANK)]
    with tc.tile_pool(name="na", bufs=3) as na, \
         tc.tile_pool(name="napsum", bufs=2, space="PSUM") as nap:
        for t in range(cfg.TILES):
            xt = na.tile([P, cfg.KCH, P], bf16, tag="xt")
            for k in range(cfg.KCH):
                nc.sync.dma_start(out=xt[:, k, :],
                                  in_=xT[k, :, t * P:(t + 1) * P])
            ph = nap.tile([WA, P], f32, tag="ph")
            for k in range(cfg.KCH):
                nc.tensor.matmul(out=ph[:], lhsT=wall_sb[:, k, :],
                                 rhs=xt[:, k, :],
                                 start=(k == 0), stop=(k == cfg.KCH - 1))
            hT = na.tile([WA, P], f32, tag="hT")
            nc.scalar.copy(out=hT[:], in_=ph[:])
            pr = nap.tile([P, WA], f32, tag="pr")
            nc.tensor.transpose(out=pr[:], in_=hT[:],
                                identity=ident[:WA, :WA])
            row = na.tile([P, cfg.ROW1], bf16, tag="row")
            nc.vector.memset(row[:, WA:], 0.0)
            nc.scalar.copy(out=row[:, :WA], in_=pr[:])
            nc.vector.tensor_copy(
                out=ad1_buf[:, t * H1:(t + 1) * H1],
                in_=pr[:, D1 + H1:D1 + 2 * H1])
            nc.vector.tensor_copy(
                out=sl1_buf[:, t * WA:(t + 1) * WA], in_=pr[:])
            nc.sync.dma_start(out=t1loc[t * P:(t + 1) * P, :], in_=row[:])
            if SPLIT_AG:
                for b in range(cfg.NBANK):
                    if qt[b] == t:
                        quarter_ag(nc, cfg, b, t1loc, t1bank[b], scr1)


def self_loops(nc, tc, cfg, layer, sl_buf, agg):
    """Initialize agg with each node's self-loop contribution:
    numerator slots = w * h, denominator slots = w + 1e-16 where
    w = exp(leaky_relu(alpha_s + alpha_d))."""
    f32 = mybir.dt.float32
    H = cfg.H1 if layer == 1 else 1
    D = cfg.D1 if layer == 1 else cfg.C
    O = cfg.O1 if layer == 1 else cfg.C
    WB = D + 2 * H                       # row width in sl_buf
    fcols = cfg.F1 if layer == 1 else cfg.F2
    with tc.tile_pool(name=f"sl{layer}", bufs=3) as sp:
        for t in range(cfg.TILES):
            base = t * WB
            w = sp.tile([P, H], f32, tag="w")
            nc.vector.tensor_tensor(
                out=w[:], in0=sl_buf[:, base + D:base + D + H],
                in1=sl_buf[:, base + D + H:base + D + 2 * H],
                op=mybir.AluOpType.add)
            nc.scalar.activation(out=w[:], in_=w[:],
                                 func=mybir.ActivationFunctionType.Prelu,
                                 alpha=cfg.NEG)
            nc.scalar.activation(out=w[:], in_=w[:],
                                 func=mybir.ActivationFunctionType.Exp)
            nc.vector.tensor_tensor(
                out=agg[:, t * fcols:t * fcols + D].rearrange(
                    "p (h o) -> p h o", h=H),
                in0=sl_buf[:, base:base + D].rearrange(
                    "p (h o) -> p h o", h=H),
                in1=w[:].unsqueeze(2).to_broadcast([P, H, O]),
                op=mybir.AluOpType.mult)
            nc.vector.tensor_scalar_add(
                agg[:, t * fcols + D:t * fcols + D + H], w[:], 1e-16)


def node_phase2(nc, tc, cfg, agg1, b1_sb, w2aug_sb, ident, ad2_buf, sl2_buf,
                t2loc, t2bank, scr2):
    f32 = mybir.dt.float32
    H1, D1, O1, C, F1 = cfg.H1, cfg.D1, cfg.O1, cfg.C, cfg.F1
    qt = [(cfg.QUART * (b + 1) + P - 1) // P - 1 for b in range(cfg.NBANK)]
    with tc.tile_pool(name="nb", bufs=3) as nb, \
         tc.tile_pool(name="nbpsum", bufs=2, space="PSUM") as nbp:
        for t in range(cfg.TILES):
            rec = nb.tile([P, H1], f32, tag="rec")
            nc.vector.reciprocal(
                out=rec[:], in_=agg1[:, t * F1 + D1:t * F1 + D1 + H1])
            o1 = nb.tile([P, D1], f32, tag="o1")
            nc.vector.tensor_tensor(
                out=o1[:].rearrange("p (h o) -> p h o", h=H1),
                in0=agg1[:, t * F1:t * F1 + D1].rearrange(
                    "p (h o) -> p h o", h=H1),
                in1=rec[:].unsqueeze(2).to_broadcast([P, H1, O1]),
                op=mybir.AluOpType.mult)
            nc.vector.tensor_add(out=o1[:], in0=o1[:], in1=b1_sb[:])
            # elu
            eneg = nb.tile([P, D1], f32, tag="eneg")
            nc.vector.tensor_scalar_min(eneg[:], o1[:], 0.0)
            nc.scalar.activation(out=eneg[:], in_=eneg[:],
                                 func=mybir.ActivationFunctionType.Exp)
            h = nb.tile([P, D1], f32, tag="h")
            nc.vector.tensor_scalar_max(h[:], o1[:], 0.0)
            nc.vector.tensor_add(out=h[:], in0=h[:], in1=eneg[:])
            nc.vector.tensor_scalar_add(h[:], h[:], -1.0)
            # h2 = [elu] @ w2aug via two PE transposes
            phT = nbp.tile([D1, P], f32, tag="phT")
            nc.tensor.transpose(out=phT[:], in_=h[:], identity=ident[:])
            hT2 = nb.tile([D1, P], f32, tag="hT2")
            nc.scalar.copy(out=hT2[:], in_=phT[:])
            p2T = nbp.tile([C + 2, P], f32, tag="p2T")
            nc.tensor.matmul(out=p2T[:], lhsT=w2aug_sb[:], rhs=hT2[:],
                             start=True, stop=True)
            h2T = nb.tile([C + 2, P], f32, tag="h2T")
            nc.scalar.copy(out=h2T[:], in_=p2T[:])
            p2 = nbp.tile([P, C + 2], f32, tag="p2")
            nc.tensor.transpose(out=p2[:], in_=h2T[:],
                                identity=ident[:C + 2, :C + 2])
            row2 = nb.tile([P, cfg.ROW2], f32, tag="row2")
            nc.vector.memset(row2[:, C + 2:], 0.0)
            nc.scalar.copy(out=row2[:, :C + 2], in_=p2[:])
            nc.vector.tensor_copy(out=ad2_buf[:, t:t + 1],
                                  in_=p2[:, C + 1:C + 2])
            nc.vector.tensor_copy(
                out=sl2_buf[:, t * (C + 2):(t + 1) * (C + 2)], in_=p2[:])
            nc.sync.dma_start(out=t2loc[t * P:(t + 1) * P, :], in_=row2[:])
            if SPLIT_AG:
                for b in range(cfg.NBANK):
                    if qt[b] == t:
                        quarter_ag(nc, cfg, b, t2loc, t2bank[b], scr2)


def epilogue(nc, tc, cfg, agg2, b2_sb, o2st, sst, lnst, out_d):
    """Three passes so the scalar engine never alternates Exp/Ln act
    tables (a table reload costs 1.3us): per-tile Exp+rowsum into sst,
    one batched Ln, per-tile subtract + output DMA."""
    f32 = mybir.dt.float32
    C, F2 = cfg.C, cfg.F2
    with tc.tile_pool(name="ep", bufs=4) as ep:
        for t in range(cfg.TILES):
            rec = ep.tile([P, 1], f32, tag="rec2")
            nc.vector.reciprocal(
                out=rec[:], in_=agg2[:, t * F2 + C:t * F2 + C + 1])
            o2 = o2st[:, t * C:(t + 1) * C]
            nc.vector.tensor_tensor(
                out=o2, in0=agg2[:, t * F2:t * F2 + C],
                in1=rec[:].to_broadcast([P, C]),
                op=mybir.AluOpType.mult)
            nc.vector.tensor_add(out=o2, in0=o2, in1=b2_sb[:])
            exps = ep.tile([P, C], f32, tag="exps")
            nc.scalar.activation(out=exps[:], in_=o2,
                                 func=mybir.ActivationFunctionType.Exp,
                                 accum_out=sst[:, t:t + 1])
        nc.scalar.activation(out=lnst[:], in_=sst[:],
                             func=mybir.ActivationFunctionType.Ln)
        for t in range(cfg.TILES):
            fin = ep.tile([P, C], f32, tag="fin")
            nc.vector.tensor_tensor(
                out=fin[:], in0=o2st[:, t * C:(t + 1) * C],
                in1=lnst[:, t:t + 1].to_broadcast([P, C]),
                op=mybir.AluOpType.subtract)
            nc.sync.dma_start(out=out_d[t * P:(t + 1) * P, :], in_=fin[:])


def edge_phase(nc, tc, cfg, meta, layer, banks, row_elems, fcols, gdt,
               gidx_d, drelc_d, drelf_d, iotar_sb, iotac_sb, ones1_sb,
               ad_buf, agg):
    f32, bf16, i16 = mybir.dt.float32, mybir.dt.bfloat16, mybir.dt.int16
    H = cfg.H1 if layer == 1 else 1
    D = cfg.D1 if layer == 1 else cfg.C          # message feature count
    O = cfg.O1 if layer == 1 else cfg.C          # feats per head
    asl_lo = D                                   # alpha_src col within row
    BC = BATCH_CHUNKS
    nb_seen = 0          # drelf rows are shared by both layers

    with tc.tile_pool(name=f"eg{layer}", bufs=4) as eg, \
         tc.tile_pool(name=f"em{layer}", bufs=4) as em, \
         tc.tile_pool(name=f"epr{layer}", bufs=2, space="PSUM") as epr, \
         tc.tile_pool(name=f"epa{layer}", bufs=2, space="PSUM") as epa, \
         tc.tile_pool(name=f"epd{layer}", bufs=2, space="PSUM") as epd:
        psum_agg = None
        for b in range(cfg.NBANK):
            off = int(meta["bank_off"][b])
            tbl_bank = banks[b]
            for (lo, hi) in meta["batches"][b]:
                nchb = hi - lo
                idx_t = em.tile([P, BC * 8], i16, tag="idx")
                nc.sync.dma_start(
                    out=idx_t[:, :nchb * 8],
                    in_=gidx_d[:, (off + lo) * 8:(off + hi) * 8])
                drc_t = em.tile([P, BC], bf16, tag="drc")
                drf_t = em.tile([1, BC * P], bf16, tag="drf")
                nc.sync.dma_start(out=drc_t[:, :nchb],
                                  in_=drelc_d[:, off + lo:off + hi])
                nc.sync.dma_start(out=drf_t[:], in_=drelf_d[nb_seen])
                g = eg.tile([P, BC, row_elems], gdt, tag="g")
                # WAR-dep anchor for the gather's overwrite of g: a tiny
                # write on the Vector engine (not Pool, keeping the gather
                # critical path clear) that the framework orders after the
                # previous batch's reads of this buffer.
                nc.vector.memset(g[0:1, 0:1, 0:4], 0.0)
                nc.gpsimd.dma_gather(
                    out_ap=g[:, :nchb, :], in_ap=tbl_bank,
                    idxs_ap=idx_t[:, :nchb * 8], num_idxs=nchb * P,
                    num_idxs_reg=nchb * P, elem_size=row_elems)
                nb_seen += 1

                # selection matrices + alpha_dst expansion, per 4-chunk group
                st = em.tile([P, BC, P], bf16, tag="st")
                pad = epd.tile([P, BC * H], f32, tag="pad")
                for gi in range(nchb // 4):
                    c0 = 4 * gi
                    # spre: per-edge drel broadcast to all partitions (PSUM)
                    pr1 = epr.tile([P, 512], f32, tag="pr1")
                    nc.tensor.matmul(out=pr1[:], lhsT=ones1_sb[:],
                                     rhs=drf_t[0:1, gi * 512:(gi + 1) * 512],
                                     start=True, stop=True)
                    # s[d, c, j] = (drel(c,j) == d)   (S^T, dst on partitions)
                    s = em.tile([P, 4, P], bf16, tag="s")
                    nc.vector.tensor_tensor(
                        out=s[:],
                        in0=pr1[:].rearrange("p (a b) -> p a b", a=4),
                        in1=iotac_sb[:].unsqueeze(1).to_broadcast([P, 4, P]),
                        op=mybir.AluOpType.is_equal)
                    # st[e, c, j] = (drel(c,e) == j)  (S, edges on partitions)
                    nc.vector.tensor_tensor(
                        out=st[:, c0:c0 + 4, :],
                        in0=drc_t[:, c0:c0 + 4].unsqueeze(2).to_broadcast(
                            [P, 4, P]),
                        in1=iotar_sb[:].unsqueeze(1).to_broadcast([P, 4, P]),
                        op=mybir.AluOpType.is_equal)
                    # pad[e, h] = alpha_d[drel_e, h]
                    for c in range(4):
                        t_c = meta["chunks"][b][lo + c0 + c][0]
                        nc.tensor.matmul(
                            out=pad[:, (c0 + c) * H:(c0 + c + 1) * H],
                            lhsT=s[:, c, :],
                            rhs=ad_buf[:, t_c * H:(t_c + 1) * H],
                            start=True, stop=True)
                # batch-wide: w = exp(leaky_relu(alpha_s + pad)), messages
                w = em.tile([P, BC, H], f32, tag="w")
                nc.vector.tensor_tensor(
                    out=w[:, :nchb, :],
                    in0=g[:, :nchb, asl_lo:asl_lo + H],
                    in1=pad[:, :nchb * H].rearrange("p (a b) -> p a b",
                                                    b=H),
                    op=mybir.AluOpType.add)
                nc.scalar.activation(
                    out=w[:, :nchb, :], in_=w[:, :nchb, :],
                    func=mybir.ActivationFunctionType.Prelu, alpha=cfg.NEG)
                nc.scalar.activation(
                    out=w[:, :nchb, :], in_=w[:, :nchb, :],
                    func=mybir.ActivationFunctionType.Exp)
                m = em.tile([P, BC, fcols], bf16, tag="m")
                nc.vector.tensor_tensor(
                    out=m[:, :nchb, :D].rearrange(
                        "p a (h o) -> p a h o", h=H),
                    in0=g[:, :nchb, :D].rearrange(
                        "p a (h o) -> p a h o", h=H),
                    in1=w[:, :nchb, :].unsqueeze(3).to_broadcast(
                        [P, nchb, H, O]),
                    op=mybir.AluOpType.mult)
                nc.scalar.copy(out=m[:, :nchb, D:D + H], in_=w[:, :nchb, :])
                # aggregate chunks into PSUM runs, flush on stop
                for c in range(nchb):
                    t_c, start_c, stop_c = meta["chunks"][b][lo + c]
                    if start_c:
                        psum_agg = epa.tile([P, fcols], f32, tag="agg")
                    nc.tensor.matmul(out=psum_agg[:], lhsT=st[:, c, :],
                                     rhs=m[:, c, :],
                                     start=start_c, stop=stop_c)
                    if stop_c:
                        nc.vector.tensor_tensor(
                            out=agg[:, t_c * fcols:(t_c + 1) * fcols],
                            in0=agg[:, t_c * fcols:(t_c + 1) * fcols],
                            in1=psum_agg[:], op=mybir.AluOpType.add)


# ------------------------------------------------------------------ kernel

_CACHE = {}


def get_program(cfg, meta):
    schedule_sig = tuple(
        tuple(meta["chunks"][b]) for b in range(cfg.NBANK))
    key = ("full", SPLIT_AG, BATCH_CHUNKS, meta["nch_tot"],
           hash(schedule_sig))
    if key not in _CACHE:
        _CACHE[key] = build_program(cfg, meta)
    return _CACHE[key]


def kernel(**inputs):
    cfg = FULL
    x = np.asarray(inputs["x"], np.float32)
    ei = np.asarray(inputs["edge_index"])
    W1 = np.asarray(inputs["W1"], np.float32)
    a_s1 = np.asarray(inputs["att_src1"], np.float32)
    a_d1 = np.asarray(inputs["att_dst1"], np.float32)
    b1 = np.asarray(inputs["bias1"], np.float32)
    W2 = np.asarray(inputs["W2"], np.float32)
    a_s2 = np.asarray(inputs["att_src2"], np.float32)
    a_d2 = np.asarray(inputs["att_dst2"], np.float32)
    b2 = np.asarray(inputs["bias2"], np.float32)

    src = ei[0].astype(np.int64)
    dst = ei[1].astype(np.int64)

    meta, gidx_all, drel_all = build_edge_meta(cfg, src, dst)
    nc = get_program(cfg, meta)

    in_maps = []
    for c in range(NCORES):
        in_maps.append(prep_core_inputs(
            cfg, meta, c, x, W1, a_s1, a_d1, b1, W2, a_s2, a_d2, b2,
            gidx_all[c], drel_all[c]))
    res = run_bass_kernel_spmd(nc, in_maps, list(range(NCORES)))
    outs = [res.results[c]["out"][: cfg.NPC] for c in range(NCORES)]
    return np.concatenate(outs, axis=0)[: cfg.N].astype(np.float32)



# revision 5
# speedup vs baseline: 12.3099x; 12.3099x over previous
"""Trainium2 Bass kernel for a 2-layer GAT (GNN message passing).

Strategy (8 NeuronCores, SPMD, single launch):
  - Destination-shard nodes: core c owns dst nodes [c*12500, (c+1)*12500).
    Each core receives all edges into its nodes -> segment softmax needs no
    cross-core reduction.
  - Node phase 1 on each core: h1 = x_slice @ [W1 | W1@A_s | W1@A_d] on PE,
    rows [h1(64) | alpha_s(8) | alpha_d(8)] stored as 256B bf16 rows.
  - Table AllGather is split into 4 quarter collectives (one per gather
    bank) so edge processing of bank b starts as soon as quarter b lands.
  - Edge phase (bank-major): dma_gather fetches per-edge src rows in
    4096-index batches (int16 indices relative to one of 4 banks of
    25088 rows).  Per 128-edge chunk, segment aggregation is a PE matmul
    with a selection matrix built from an iota compare; alpha_dst is
    expanded per-edge with the transposed selection matrix (built by a
    DVE compare directly against a PE-broadcast PSUM row); softmax
    denominators ride along as extra matmul columns; the division is
    deferred to a per-node post-scale.
  - Self-loop edges are *not* gathered: their contribution (w=exp(lrelu(
    alpha_s+alpha_d)) to numerator+denominator) is computed node-locally
    per tile, which also initializes the aggregation buffers.
  - Node phase 2: divide, +bias, ELU, @ [W2 | folded attention vectors],
    quarter AllGathers, edge phase 2 (identical edge schedule), epilogue
    (divide, bias, log-softmax).
"""

import sys

sys.path.insert(0, "/opt/trn_rl_repo")

import numpy as np
import ml_dtypes

import concourse.bass as bass
import concourse.bacc as bacc
import concourse.mybir as mybir
from concourse.tile import TileContext
from concourse.bass_utils import run_bass_kernel_spmd

import os

BF16 = ml_dtypes.bfloat16
P = 128
NCORES = 8
# chunks per dma_gather call (x128 = indices per call).  The SWDGE
# descriptor rings overflow (ring-space wait deadlocks) for larger
# calls; 8 chunks (1024 indices, 65 descriptors/ring) is validated.
BATCH_CHUNKS = int(os.environ.get("BATCH_CHUNKS", "8"))
SPLIT_AG = int(os.environ.get("SPLIT_AG", "0"))   # 4 quarter-AGs vs 1 full

# ---------------------------------------------------------------- config


class Cfg:
    def __init__(self, n_nodes, n_edges, f_in, heads1, out1, n_classes,
                 npc, nbank, neg_slope=0.2):
        self.N = n_nodes
        self.E = n_edges
        self.F_IN = f_in                    # 256
        self.H1 = heads1                    # 8
        self.O1 = out1                      # 8
        self.C = n_classes                  # 40
        self.NEG = neg_slope
        self.NPC = npc                      # raw nodes per core
        assert npc * NCORES >= n_nodes
        self.TILES = (npc + P - 1) // P
        self.NPAD = self.TILES * P          # padded nodes per core
        self.NTOT = NCORES * self.NPAD      # table rows
        self.NBANK = nbank
        assert self.NPAD % nbank == 0
        self.QUART = self.NPAD // nbank     # rows per core per bank
        self.BANK = NCORES * self.QUART     # rows per bank
        assert self.BANK <= 32768
        self.D1 = heads1 * out1             # 64
        self.F1 = self.D1 + heads1          # 72 (msg cols + denom cols)
        self.F2 = n_classes + 1             # 41
        self.ROW1 = 128                     # bf16 elems/row in table1 (256B)
        self.ROW2 = 64                      # fp32 elems/row in table2 (256B)
        assert self.D1 + 2 * heads1 <= self.ROW1
        assert n_classes + 2 <= self.ROW2
        self.KCH = (f_in + P - 1) // P      # k-chunks in node matmul 1


FULL = Cfg(n_nodes=100000, n_edges=1600000, f_in=256, heads1=8, out1=8,
           n_classes=40, npc=12500, nbank=4)


# ------------------------------------------------------- host preprocessing


def build_edge_meta(cfg, src, dst):
    """Partition/sort/pad (non-self-loop) edges.

    Table row layout (matches the 4 quarter-AllGathers): node n with
    core c = n // NPC, local l = n % NPC sits in bank q = l // QUART at
    bank-index c * QUART + (l % QUART).

    Returns (meta, per-core idx/drel streams).  meta is identical across
    cores: per bank chunk list [(tile, start, stop)], 32-chunk batches,
    global chunk offsets per bank.
    """
    s_core, s_loc = src // cfg.NPC, src % cfg.NPC
    if SPLIT_AG:
        bank = s_loc // cfg.QUART
        bidx = s_core * cfg.QUART + (s_loc % cfg.QUART)
    else:
        src_row = s_core * cfg.NPAD + s_loc
        bank = src_row // cfg.BANK
        bidx = src_row % cfg.BANK
    dst_core = dst // cfg.NPC
    dst_loc = dst % cfg.NPC
    tile = dst_loc // P
    drel = dst_loc % P

    counts = np.zeros((NCORES, cfg.NBANK, cfg.TILES), np.int64)
    np.add.at(counts, (dst_core, bank, tile), 1)
    K = np.ceil(counts.max(axis=0) / P).astype(np.int64)      # [NBANK, TILES]

    # pad each bank's chunk count to a multiple of 4 (group granularity)
    for b in range(cfg.NBANK):
        tot = int(K[b].sum())
        extra = (-tot) % 4
        if extra and tot > 0:
            tstar = int(np.nonzero(K[b])[0][-1])
            K[b, tstar] += extra

    chunks = []          # per bank: list of (tile, start, stop)
    batches = []         # per bank: list of (lo, hi)
    for b in range(cfg.NBANK):
        ch = []
        for t in range(cfg.TILES):
            k = int(K[b, t])
            for i in range(k):
                ch.append((t, i == 0, i == k - 1))
        chunks.append(ch)
        bt = []
        lo = 0
        while lo < len(ch):
            hi = min(lo + BATCH_CHUNKS, len(ch))
            bt.append((lo, hi))
            lo = hi
        batches.append(bt)

    nch_bank = [len(c) for c in chunks]
    nch_tot = sum(nch_bank)
    bank_off = np.cumsum([0] + nch_bank)[:-1]

    order_key = (dst_core * cfg.NBANK + bank) * cfg.TILES + tile
    perm = np.argsort(order_key, kind="stable")
    s_core_, s_bank, s_tile = dst_core[perm], bank[perm], tile[perm]
    s_bidx, s_drel = bidx[perm], drel[perm]

    gidx_all = np.zeros((NCORES, nch_tot * P), np.int16)
    drel_all = np.full((NCORES, nch_tot * P), -1.0, np.float32)

    run_off = np.zeros((NCORES, cfg.NBANK, cfg.TILES), np.int64)
    for b in range(cfg.NBANK):
        off = 0
        for t in range(cfg.TILES):
            run_off[:, b, t] = bank_off[b] * P + off * P
            off += int(K[b, t])
    grp = s_core_ * (cfg.NBANK * cfg.TILES) + s_bank * cfg.TILES + s_tile
    first = np.r_[True, grp[1:] != grp[:-1]]
    gstart = np.maximum.accumulate(np.where(first, np.arange(len(grp)), 0))
    within = np.arange(len(grp)) - gstart
    pos = run_off[s_core_, s_bank, s_tile] + within
    gidx_all[s_core_, pos] = s_bidx.astype(np.int16)
    drel_all[s_core_, pos] = s_drel.astype(np.float32)

    meta = dict(K=K, chunks=chunks, batches=batches, bank_off=bank_off,
                nch_tot=nch_tot)
    return meta, gidx_all, drel_all


def wrap_idx(gidx_flat):
    """idx stream [E] -> dma_gather layout [128, E/16] (16-lane wrap,
    replicated into the 8 sixteen-partition groups)."""
    e = gidx_flat.shape[0]
    assert e % 16 == 0
    w = gidx_flat.reshape(e // 16, 16).T          # [16, E/16]
    return np.tile(w, (8, 1)).astype(np.int16)     # [128, E/16]


def prep_core_inputs(cfg, meta, core, x, W1, a_s1, a_d1, b1, W2, a_s2, a_d2,
                     b2, gidx_core, drel_core):
    n0, n1 = core * cfg.NPC, min((core + 1) * cfg.NPC, cfg.N)
    xs = np.zeros((cfg.NPAD, cfg.F_IN), np.float32)
    xs[: n1 - n0] = x[n0:n1]
    xT = np.ascontiguousarray(xs.T)                          # [F_IN, NPAD]
    kch = cfg.KCH
    xT_s = np.zeros((kch, P, cfg.NPAD), BF16)
    for k in range(kch):
        lo, hi = k * P, min((k + 1) * P, cfg.F_IN)
        xT_s[k, : hi - lo] = xT[lo:hi].astype(BF16)

    A_s = np.zeros((cfg.D1, cfg.H1), np.float32)
    A_d = np.zeros((cfg.D1, cfg.H1), np.float32)
    for h in range(cfg.H1):
        A_s[h * cfg.O1:(h + 1) * cfg.O1, h] = a_s1[h]
        A_d[h * cfg.O1:(h + 1) * cfg.O1, h] = a_d1[h]
    Wfull = np.concatenate([W1, W1 @ A_s, W1 @ A_d], axis=1)  # [F_IN, 80]
    wall = np.zeros((kch, P, cfg.D1 + 2 * cfg.H1), BF16)
    for k in range(kch):
        lo, hi = k * P, min((k + 1) * P, cfg.F_IN)
        wall[k, : hi - lo] = Wfull[lo:hi].astype(BF16)

    w2aug = np.concatenate(
        [W2, (W2 @ a_s2[0])[:, None], (W2 @ a_d2[0])[:, None]], axis=1
    ).astype(np.float32)

    bias1r = np.tile(b1[None, :], (P, 1)).astype(np.float32)
    bias2r = np.tile(b2[None, :], (P, 1)).astype(np.float32)
    iotar = np.tile(np.arange(P, dtype=np.float32)[None, :], (P, 1)).astype(BF16)
    iotac = np.tile(np.arange(P, dtype=np.float32)[:, None], (1, P)).astype(BF16)
    ones1 = np.ones((1, P), BF16)
    identm = np.eye(P, dtype=np.float32)

    nch = meta["nch_tot"]
    gidx = wrap_idx(gidx_core)                               # [128, nch*8]
    drelc = np.ascontiguousarray(
        drel_core.reshape(nch, P).T).astype(BF16)            # [128, nch]
    rows = []
    for b in range(cfg.NBANK):
        off = meta["bank_off"][b]
        for (lo, hi) in meta["batches"][b]:
            r = np.full((BATCH_CHUNKS * P,), -1.0, np.float32)
            r[: (hi - lo) * P] = drel_core[(off + lo) * P:(off + hi) * P]
            rows.append(r[None, :])
    drelf = (np.stack(rows).astype(BF16) if rows
             else np.zeros((1, 1, BATCH_CHUNKS * P), BF16))

    return dict(xT=xT_s, wall=wall, w2aug=w2aug, bias1r=bias1r, bias2r=bias2r,
                iotar=iotar, iotac=iotac, ones1=ones1, identd=identm,
                gidx=gidx, drelc=drelc, drelf=drelf)


# ------------------------------------------------------------ bass program


def build_program(cfg, meta, phases="ACDFG", skip_coll=False,
                  init_missing=False):
    nc = bacc.Bacc(None, target_bir_lowering=False, debug=False)
    f32, bf16, i16 = mybir.dt.float32, mybir.dt.bfloat16, mybir.dt.int16

    nch = meta["nch_tot"]
    nbatch_tot = sum(len(b) for b in meta["batches"])

    xT = nc.declare_dram_parameter("xT", [cfg.KCH, P, cfg.NPAD], bf16, isOutput=False)
    wall = nc.declare_dram_parameter("wall", [cfg.KCH, P, cfg.D1 + 2 * cfg.H1], bf16, isOutput=False)
    w2aug = nc.declare_dram_parameter("w2aug", [cfg.D1, cfg.C + 2], f32, isOutput=False)
    bias1r = nc.declare_dram_parameter("bias1r", [P, cfg.D1], f32, isOutput=False)
    bias2r = nc.declare_dram_parameter("bias2r", [P, cfg.C], f32, isOutput=False)
    identd = nc.declare_dram_parameter("identd", [P, P], f32, isOutput=False)
    iotar_d = nc.declare_dram_parameter("iotar", [P, P], bf16, isOutput=False)
    iotac_d = nc.declare_dram_parameter("iotac", [P, P], bf16, isOutput=False)
    ones1_d = nc.declare_dram_parameter("ones1", [1, P], bf16, isOutput=False)
    gidx_d = nc.declare_dram_parameter("gidx", [P, nch * 8], i16, isOutput=False)
    drelc_d = nc.declare_dram_parameter("drelc", [P, nch], bf16, isOutput=False)
    drelf_d = nc.declare_dram_parameter("drelf", [nbatch_tot, 1, BATCH_CHUNKS * P], bf16, isOutput=False)
    out_d = nc.declare_dram_parameter("out", [cfg.NPAD, cfg.C], f32, isOutput=True)

    t1loc = nc.dram_tensor("t1loc", [cfg.NPAD, cfg.ROW1], bf16)
    t2loc = nc.dram_tensor("t2loc", [cfg.NPAD, cfg.ROW2], f32)
    if SPLIT_AG:
        t1bank = [nc.dram_tensor(f"t1bank{b}", [cfg.BANK, cfg.ROW1], bf16,
                                 addr_space="Shared") for b in range(cfg.NBANK)]
        t2bank = [nc.dram_tensor(f"t2bank{b}", [cfg.BANK, cfg.ROW2], f32,
                                 addr_space="Shared") for b in range(cfg.NBANK)]
        t1bank_ap = [h[:] for h in t1bank]
        t2bank_ap = [h[:] for h in t2bank]
    else:
        t1full = nc.dram_tensor("t1full", [cfg.NTOT, cfg.ROW1], bf16,
                                addr_space="Shared")
        t2full = nc.dram_tensor("t2full", [cfg.NTOT, cfg.ROW2], f32,
                                addr_space="Shared")
        t1bank_ap = [t1full[b * cfg.BANK:(b + 1) * cfg.BANK, :]
                     for b in range(cfg.NBANK)]
        t2bank_ap = [t2full[b * cfg.BANK:(b + 1) * cfg.BANK, :]
                     for b in range(cfg.NBANK)]
        t1bank = t2bank = None

    H1, D1, C = cfg.H1, cfg.D1, cfg.C
    F1, F2 = cfg.F1, cfg.F2
    WA = D1 + 2 * H1                                   # 80

    with TileContext(nc) as tc:
        with tc.tile_pool(name="persist", bufs=1) as pp:
            ident = pp.tile([P, P], f32)
            nc.sync.dma_start(out=ident[:], in_=identd[:])
            wall_sb = pp.tile([P, cfg.KCH, WA], bf16)
            for k in range(cfg.KCH):
                nc.sync.dma_start(out=wall_sb[:, k, :], in_=wall[k])
            w2aug_sb = pp.tile([D1, C + 2], f32)
            nc.sync.dma_start(out=w2aug_sb[:], in_=w2aug[:])
            b1_sb = pp.tile([P, D1], f32)
            nc.sync.dma_start(out=b1_sb[:], in_=bias1r[:])
            b2_sb = pp.tile([P, C], f32)
            nc.sync.dma_start(out=b2_sb[:], in_=bias2r[:])
            iotar_sb = pp.tile([P, P], bf16)
            nc.sync.dma_start(out=iotar_sb[:], in_=iotar_d[:])
            iotac_sb = pp.tile([P, P], bf16)
            nc.sync.dma_start(out=iotac_sb[:], in_=iotac_d[:])
            ones1_sb = pp.tile([1, P], bf16)
            nc.sync.dma_start(out=ones1_sb[:], in_=ones1_d[:])
            ad1_buf = pp.tile([P, cfg.TILES * H1], bf16)
            ad2_buf = pp.tile([P, cfg.TILES], bf16)
            sl1_buf = pp.tile([P, cfg.TILES * WA], bf16)
            sl2_buf = pp.tile([P, cfg.TILES * (C + 2)], bf16)
            agg1 = pp.tile([P, cfg.TILES * F1], f32)
            agg2 = pp.tile([P, cfg.TILES * F2], f32)
            o2st = pp.tile([P, cfg.TILES * C], f32)
            sst = pp.tile([P, cfg.TILES], f32)
            lnst = pp.tile([P, cfg.TILES], f32)
            scr1 = pp.tile([cfg.NBANK, cfg.ROW1], bf16)
            scr2 = pp.tile([cfg.NBANK, cfg.ROW2], f32)

            if init_missing:
                if "A" not in phases:
                    nc.vector.memset(sl1_buf[:], 0.0)
                    nc.vector.memset(ad1_buf[:], 0.0)
                if "C" not in phases and ("D" in phases or "G" in phases):
                    nc.vector.memset(agg1[:], 1.0)
                if "D" not in phases:
                    nc.vector.memset(sl2_buf[:], 0.0)
                    nc.vector.memset(ad2_buf[:], 0.0)
                if "F" not in phases and "G" in phases:
                    nc.vector.memset(agg2[:], 1.0)

            # ---------------- phase A: node transform layer 1 (+ quarter AGs)
            if "A" in phases:
                node_phase1(nc, tc, cfg, xT, wall_sb, ident, ad1_buf,
                            sl1_buf, t1loc, t1bank, scr1)
                if skip_coll:
                    nc.sync.dma_start(out=t1full[:cfg.NPAD, :], in_=t1loc[:])
                    nc.gpsimd.dma_start(
                        out=scr1[0:1, :],
                        in_=t1full[cfg.NPAD - 1:cfg.NPAD, :])
                elif not SPLIT_AG:
                    nc.gpsimd.collective_compute(
                        "AllGather", mybir.AluOpType.bypass,
                        replica_groups=[list(range(NCORES))],
                        ins=[t1loc[:]], outs=[t1full[:]])
                    nc.gpsimd.dma_start(
                        out=scr1[0:1, :],
                        in_=t1full[cfg.NTOT - 1:cfg.NTOT, :])

            # ---------------- phase C: self-loops + edge layer 1
            if "C" in phases:
                self_loops(nc, tc, cfg, layer=1, sl_buf=sl1_buf, agg=agg1)
                edge_phase(nc, tc, cfg, meta, layer=1, banks=t1bank_ap,
                           row_elems=cfg.ROW1, fcols=F1, gdt=bf16,
                           gidx_d=gidx_d, drelc_d=drelc_d, drelf_d=drelf_d,
                           iotar_sb=iotar_sb, iotac_sb=iotac_sb,
                           ones1_sb=ones1_sb, ad_buf=ad1_buf, agg=agg1)

            # ---------------- phase D: node transform layer 2 (+ quarter AGs)
            if "D" in phases:
                node_phase2(nc, tc, cfg, agg1, b1_sb, w2aug_sb, ident,
                            ad2_buf, sl2_buf, t2loc, t2bank, scr2)
                if skip_coll:
                    nc.sync.dma_start(out=t2full[:cfg.NPAD, :], in_=t2loc[:])
                    nc.gpsimd.dma_start(
                        out=scr2[0:1, :],
                        in_=t2full[cfg.NPAD - 1:cfg.NPAD, :])
                elif not SPLIT_AG:
                    nc.gpsimd.collective_compute(
                        "AllGather", mybir.AluOpType.bypass,
                        replica_groups=[list(range(NCORES))],
                        ins=[t2loc[:]], outs=[t2full[:]])
                    nc.gpsimd.dma_start(
                        out=scr2[0:1, :],
                        in_=t2full[cfg.NTOT - 1:cfg.NTOT, :])

            # ---------------- phase F: self-loops + edge layer 2
            if "F" in phases:
                self_loops(nc, tc, cfg, layer=2, sl_buf=sl2_buf, agg=agg2)
                edge_phase(nc, tc, cfg, meta, layer=2, banks=t2bank_ap,
                           row_elems=cfg.ROW2, fcols=F2, gdt=f32,
                           gidx_d=gidx_d, drelc_d=drelc_d, drelf_d=drelf_d,
                           iotar_sb=iotar_sb, iotac_sb=iotac_sb,
                           ones1_sb=ones1_sb, ad_buf=ad2_buf, agg=agg2)

            # ---------------- phase G: epilogue (divide, bias, log_softmax)
            if "G" in phases:
                epilogue(nc, tc, cfg, agg2, b2_sb, o2st, sst, lnst, out_d)

    nc.compile()
    return nc


def quarter_ag(nc, cfg, b, loc, bank_t, scr):
    """AllGather quarter b of the local table into bank b (+ dummy read
    to anchor collective completion for the tile framework)."""
    q = cfg.QUART
    nc.gpsimd.collective_compute(
        "AllGather", mybir.AluOpType.bypass,
        replica_groups=[list(range(NCORES))],
        ins=[loc[b * q:(b + 1) * q, :]], outs=[bank_t[:]])
    nc.gpsimd.dma_start(out=scr[b:b + 1, :],
                        in_=bank_t[cfg.BANK - 1:cfg.BANK, :])


def node_phase1(nc, tc, cfg, xT, wall_sb, ident, ad1_buf, sl1_buf, t1loc,
                t1bank, scr1):
    f32, bf16 = mybir.dt.float32, mybir.dt.bfloat16
    H1, D1 = cfg.H1, cfg.D1
    WA = D1 + 2 * H1
    qt = [(cfg.QUART * (b + 1) + P - 1) // P - 1 for b in range(cfg.NBANK)]
    with tc.tile_pool(name="na", bufs=3) as na, \
         tc.tile_pool(name="napsum", bufs=2, space="PSUM") as nap:
        for t in range(cfg.TILES):
            xt = na.tile([P, cfg.KCH, P], bf16, tag="xt")
            for k in range(cfg.KCH):
                nc.sync.dma_start(out=xt[:, k, :],
                                  in_=xT[k, :, t * P:(t + 1) * P])
            ph = nap.tile([WA, P], f32, tag="ph")
            for k in range(cfg.KCH):
                nc.tensor.matmul(out=ph[:], lhsT=wall_sb[:, k, :],
                                 rhs=xt[:, k, :],
                                 start=(k == 0), stop=(k == cfg.KCH - 1))
            hT = na.tile([WA, P], f32, tag="hT")
            nc.scalar.copy(out=hT[:], in_=ph[:])
            pr = nap.tile([P, WA], f32, tag="pr")
            nc.tensor.transpose(out=pr[:], in_=hT[:],
                                identity=ident[:WA, :WA])
            row = na.tile([P, cfg.ROW1], bf16, tag="row")
            nc.vector.memset(row[:, WA:], 0.0)
            nc.scalar.copy(out=row[:, :WA], in_=pr[:])
            nc.vector.tensor_copy(
                out=ad1_buf[:, t * H1:(t + 1) * H1],
                in_=pr[:, D1 + H1:D1 + 2 * H1])
            nc.vector.tensor_copy(
                out=sl1_buf[:, t * WA:(t + 1) * WA], in_=pr[:])
            nc.sync.dma_start(out=t1loc[t * P:(t + 1) * P, :], in_=row[:])
            if SPLIT_AG:
                for b in range(cfg.NBANK):
                    if qt[b] == t:
                        quarter_ag(nc, cfg, b, t1loc, t1bank[b], scr1)


def self_loops(nc, tc, cfg, layer, sl_buf, agg):
    """Initialize agg with each node's self-loop contribution:
    numerator slots = w * h, denominator slots = w + 1e-16 where
    w = exp(leaky_relu(alpha_s + alpha_d))."""
    f32 = mybir.dt.float32
    H = cfg.H1 if layer == 1 else 1
    D = cfg.D1 if layer == 1 else cfg.C
    O = cfg.O1 if layer == 1 else cfg.C
    WB = D + 2 * H                       # row width in sl_buf
    fcols = cfg.F1 if layer == 1 else cfg.F2
    with tc.tile_pool(name=f"sl{layer}", bufs=3) as sp:
        for t in range(cfg.TILES):
            base = t * WB
            w = sp.tile([P, H], f32, tag="w")
            nc.vector.tensor_tensor(
                out=w[:], in0=sl_buf[:, base + D:base + D + H],
                in1=sl_buf[:, base + D + H:base + D + 2 * H],
                op=mybir.AluOpType.add)
            nc.scalar.activation(out=w[:], in_=w[:],
                                 func=mybir.ActivationFunctionType.Prelu,
                                 alpha=cfg.NEG)
            nc.scalar.activation(out=w[:], in_=w[:],
                                 func=mybir.ActivationFunctionType.Exp)
            nc.vector.tensor_tensor(
                out=agg[:, t * fcols:t * fcols + D].rearrange(
                    "p (h o) -> p h o", h=H),
                in0=sl_buf[:, base:base + D].rearrange(
                    "p (h o) -> p h o", h=H),
                in1=w[:].unsqueeze(2).to_broadcast([P, H, O]),
                op=mybir.AluOpType.mult)
            nc.vector.tensor_scalar_add(
                agg[:, t * fcols + D:t * fcols + D + H], w[:], 1e-16)


def node_phase2(nc, tc, cfg, agg1, b1_sb, w2aug_sb, ident, ad2_buf, sl2_buf,
                t2loc, t2bank, scr2):
    f32 = mybir.dt.float32
    H1, D1, O1, C, F1 = cfg.H1, cfg.D1, cfg.O1, cfg.C, cfg.F1
    qt = [(cfg.QUART * (b + 1) + P - 1) // P - 1 for b in range(cfg.NBANK)]
    with tc.tile_pool(name="nb", bufs=3) as nb, \
         tc.tile_pool(name="nbpsum", bufs=2, space="PSUM") as nbp:
        for t in range(cfg.TILES):
            rec = nb.tile([P, H1], f32, tag="rec")
            nc.vector.reciprocal(
                out=rec[:], in_=agg1[:, t * F1 + D1:t * F1 + D1 + H1])
            o1 = nb.tile([P, D1], f32, tag="o1")
            nc.vector.tensor_tensor(
                out=o1[:].rearrange("p (h o) -> p h o", h=H1),
                in0=agg1[:, t * F1:t * F1 + D1].rearrange(
                    "p (h o) -> p h o", h=H1),
                in1=rec[:].unsqueeze(2).to_broadcast([P, H1, O1]),
                op=mybir.AluOpType.mult)
            nc.vector.tensor_add(out=o1[:], in0=o1[:], in1=b1_sb[:])
            # elu
            eneg = nb.tile([P, D1], f32, tag="eneg")
            nc.vector.tensor_scalar_min(eneg[:], o1[:], 0.0)
            nc.scalar.activation(out=eneg[:], in_=eneg[:],
                                 func=mybir.ActivationFunctionType.Exp)
            h = nb.tile([P, D1], f32, tag="h")
            nc.vector.tensor_scalar_max(h[:], o1[:], 0.0)
            nc.vector.tensor_add(out=h[:], in0=h[:], in1=eneg[:])
            nc.vector.tensor_scalar_add(h[:], h[:], -1.0)
            # h2 = [elu] @ w2aug via two PE transposes
            phT = nbp.tile([D1, P], f32, tag="phT")
            nc.tensor.transpose(out=phT[:], in_=h[:], identity=ident[:])
            hT2 = nb.tile([D1, P], f32, tag="hT2")
            nc.scalar.copy(out=hT2[:], in_=phT[:])
            p2T = nbp.tile([C + 2, P], f32, tag="p2T")
            nc.tensor.matmul(out=p2T[:], lhsT=w2aug_sb[:], rhs=hT2[:],
                             start=True, stop=True)
            h2T = nb.tile([C + 2, P], f32, tag="h2T")
            nc.scalar.copy(out=h2T[:], in_=p2T[:])
            p2 = nbp.tile([P, C + 2], f32, tag="p2")
            nc.tensor.transpose(out=p2[:], in_=h2T[:],
                                identity=ident[:C + 2, :C + 2])
            row2 = nb.tile([P, cfg.ROW2], f32, tag="row2")
            nc.vector.memset(row2[:, C + 2:], 0.0)
            nc.scalar.copy(out=row2[:, :C + 2], in_=p2[:])
            nc.vector.tensor_copy(out=ad2_buf[:, t:t + 1],
                                  in_=p2[:, C + 1:C + 2])
            nc.vector.tensor_copy(
                out=sl2_buf[:, t * (C + 2):(t + 1) * (C + 2)], in_=p2[:])
            nc.sync.dma_start(out=t2loc[t * P:(t + 1) * P, :], in_=row2[:])
            if SPLIT_AG:
                for b in range(cfg.NBANK):
                    if qt[b] == t:
                        quarter_ag(nc, cfg, b, t2loc, t2bank[b], scr2)


def epilogue(nc, tc, cfg, agg2, b2_sb, o2st, sst, lnst, out_d):
    """Three passes so the scalar engine never alternates Exp/Ln act
    tables (a table reload costs 1.3us): per-tile Exp+rowsum into sst,
    one batched Ln, per-tile subtract + output DMA."""
    f32 = mybir.dt.float32
    C, F2 = cfg.C, cfg.F2
    with tc.tile_pool(name="ep", bufs=4) as ep:
        for t in range(cfg.TILES):
            rec = ep.tile([P, 1], f32, tag="rec2")
            nc.vector.reciprocal(
                out=rec[:], in_=agg2[:, t * F2 + C:t * F2 + C + 1])
            o2 = o2st[:, t * C:(t + 1) * C]
            nc.vector.tensor_tensor(
                out=o2, in0=agg2[:, t * F2:t * F2 + C],
                in1=rec[:].to_broadcast([P, C]),
                op=mybir.AluOpType.mult)
            nc.vector.tensor_add(out=o2, in0=o2, in1=b2_sb[:])
            exps = ep.tile([P, C], f32, tag="exps")
            nc.scalar.activation(out=exps[:], in_=o2,
                                 func=mybir.ActivationFunctionType.Exp,
                                 accum_out=sst[:, t:t + 1])
        nc.scalar.activation(out=lnst[:], in_=sst[:],
                             func=mybir.ActivationFunctionType.Ln)
        for t in range(cfg.TILES):
            fin = ep.tile([P, C], f32, tag="fin")
            nc.vector.tensor_tensor(
                out=fin[:], in0=o2st[:, t * C:(t + 1) * C],
                in1=lnst[:, t:t + 1].to_broadcast([P, C]),
                op=mybir.AluOpType.subtract)
            nc.sync.dma_start(out=out_d[t * P:(t + 1) * P, :], in_=fin[:])


def edge_phase(nc, tc, cfg, meta, layer, banks, row_elems, fcols, gdt,
               gidx_d, drelc_d, drelf_d, iotar_sb, iotac_sb, ones1_sb,
               ad_buf, agg):
    f32, bf16, i16 = mybir.dt.float32, mybir.dt.bfloat16, mybir.dt.int16
    H = cfg.H1 if layer == 1 else 1
    D = cfg.D1 if layer == 1 else cfg.C          # message feature count
    O = cfg.O1 if layer == 1 else cfg.C          # feats per head
    asl_lo = D                                   # alpha_src col within row
    BC = BATCH_CHUNKS
    nb_seen = 0          # drelf rows are shared by both layers

    with tc.tile_pool(name=f"eg{layer}", bufs=4) as eg, \
         tc.tile_pool(name=f"em{layer}", bufs=4) as em, \
         tc.tile_pool(name=f"epr{layer}", bufs=2, space="PSUM") as epr, \
         tc.tile_pool(name=f"epa{layer}", bufs=2, space="PSUM") as epa, \
         tc.tile_pool(name=f"epd{layer}", bufs=2, space="PSUM") as epd:
        psum_agg = None
        for b in range(cfg.NBANK):
            off = int(meta["bank_off"][b])
            tbl_bank = banks[b]
            for (lo, hi) in meta["batches"][b]:
                nchb = hi - lo
                idx_t = em.tile([P, BC * 8], i16, tag="idx")
                nc.sync.dma_start(
                    out=idx_t[:, :nchb * 8],
                    in_=gidx_d[:, (off + lo) * 8:(off + hi) * 8])
                drc_t = em.tile([P, BC], bf16, tag="drc")
                drf_t = em.tile([1, BC * P], bf16, tag="drf")
                nc.sync.dma_start(out=drc_t[:, :nchb],
                                  in_=drelc_d[:, off + lo:off + hi])
                nc.sync.dma_start(out=drf_t[:], in_=drelf_d[nb_seen])
                g = eg.tile([P, BC, row_elems], gdt, tag="g")
                # WAR-dep anchor for the gather's overwrite of g: a tiny
                # write on the Vector engine (not Pool, keeping the gather
                # critical path clear) that the framework orders after the
                # previous batch's reads of this buffer.
                nc.vector.memset(g[0:1, 0:1, 0:4], 0.0)
                nc.gpsimd.dma_gather(
                    out_ap=g[:, :nchb, :], in_ap=tbl_bank,
                    idxs_ap=idx_t[:, :nchb * 8], num_idxs=nchb * P,
                    num_idxs_reg=nchb * P, elem_size=row_elems)
                nb_seen += 1

                # selection matrices + alpha_dst expansion, per 4-chunk group
                st = em.tile([P, BC, P], bf16, tag="st")
                pad = epd.tile([P, BC * H], f32, tag="pad")
                for gi in range(nchb // 4):
                    c0 = 4 * gi
                    # spre: per-edge drel broadcast to all partitions (PSUM)
                    pr1 = epr.tile([P, 512], f32, tag="pr1")
                    nc.tensor.matmul(out=pr1[:], lhsT=ones1_sb[:],
                                     rhs=drf_t[0:1, gi * 512:(gi + 1) * 512],
                                     start=True, stop=True)
                    # s[d, c, j] = (drel(c,j) == d)   (S^T, dst on partitions)
                    s = em.tile([P, 4, P], bf16, tag="s")
                    nc.vector.tensor_tensor(
                        out=s[:],
                        in0=pr1[:].rearrange("p (a b) -> p a b", a=4),
                        in1=iotac_sb[:].unsqueeze(1).to_broadcast([P, 4, P]),
                        op=mybir.AluOpType.is_equal)
                    # st[e, c, j] = (drel(c,e) == j)  (S, edges on partitions)
                    nc.vector.tensor_tensor(
                        out=st[:, c0:c0 + 4, :],
                        in0=drc_t[:, c0:c0 + 4].unsqueeze(2).to_broadcast(
                            [P, 4, P]),
                        in1=iotar_sb[:].unsqueeze(1).to_broadcast([P, 4, P]),
                        op=mybir.AluOpType.is_equal)
                    # pad[e, h] = alpha_d[drel_e, h]
                    for c in range(4):
                        t_c = meta["chunks"][b][lo + c0 + c][0]
                        nc.tensor.matmul(
                            out=pad[:, (c0 + c) * H:(c0 + c + 1) * H],
                            lhsT=s[:, c, :],
                            rhs=ad_buf[:, t_c * H:(t_c + 1) * H],
                            start=True, stop=True)
                # batch-wide: w = exp(leaky_relu(alpha_s + pad)), messages
                w = em.tile([P, BC, H], f32, tag="w")
                nc.vector.tensor_tensor(
                    out=w[:, :nchb, :],
                    in0=g[:, :nchb, asl_lo:asl_lo + H],
                    in1=pad[:, :nchb * H].rearrange("p (a b) -> p a b",
                                                    b=H),
                    op=mybir.AluOpType.add)
                nc.scalar.activation(
                    out=w[:, :nchb, :], in_=w[:, :nchb, :],
                    func=mybir.ActivationFunctionType.Prelu, alpha=cfg.NEG)
                nc.scalar.activation(
                    out=w[:, :nchb, :], in_=w[:, :nchb, :],
                    func=mybir.ActivationFunctionType.Exp)
                m = em.tile([P, BC, fcols], bf16, tag="m")
                nc.vector.tensor_tensor(
                    out=m[:, :nchb, :D].rearrange(
                        "p a (h o) -> p a h o", h=H),
                    in0=g[:, :nchb, :D].rearrange(
                        "p a (h o) -> p a h o", h=H),
                    in1=w[:, :nchb, :].unsqueeze(3).to_broadcast(
                        [P, nchb, H, O]),
                    op=mybir.AluOpType.mult)
                nc.scalar.copy(out=m[:, :nchb, D:D + H], in_=w[:, :nchb, :])
                # aggregate chunks into PSUM runs, flush on stop
                for c in range(nchb):
                    t_c, start_c, stop_c = meta["chunks"][b][lo + c]
                    if start_c:
                        psum_agg = epa.tile([P, fcols], f32, tag="agg")
                    nc.tensor.matmul(out=psum_agg[:], lhsT=st[:, c, :],
                                     rhs=m[:, c, :],
                                     start=start_c, stop=stop_c)
                    if stop_c:
                        nc.vector.tensor_tensor(
                            out=agg[:, t_c * fcols:(t_c + 1) * fcols],
                            in0=agg[:, t_c * fcols:(t_c + 1) * fcols],
                            in1=psum_agg[:], op=mybir.AluOpType.add)


# ------------------------------------------------------------------ kernel

_CACHE = {}


def get_program(cfg, meta):
    schedule_sig = tuple(
        tuple(meta["chunks"][b]) for b in range(cfg.NBANK))
    key = ("full", SPLIT_AG, BATCH_CHUNKS, meta["nch_tot"],
           hash(schedule_sig))
    if key not in _CACHE:
        _CACHE[key] = build_program(cfg, meta)
    return _CACHE[key]


def kernel(**inputs):
    cfg = FULL
    x = np.asarray(inputs["x"], np.float32)
    ei = np.asarray(inputs["edge_index"])
    W1 = np.asarray(inputs["W1"], np.float32)
    a_s1 = np.asarray(inputs["att_src1"], np.float32)
    a_d1 = np.asarray(inputs["att_dst1"], np.float32)
    b1 = np.asarray(inputs["bias1"], np.float32)
    W2 = np.asarray(inputs["W2"], np.float32)
    a_s2 = np.asarray(inputs["att_src2"], np.float32)
    a_d2 = np.asarray(inputs["att_dst2"], np.float32)
    b2 = np.asarray(inputs["bias2"], np.float32)

    src = ei[0].astype(np.int64)
    dst = ei[1].astype(np.int64)

    meta, gidx_all, drel_all = build_edge_meta(cfg, src, dst)
    nc = get_program(cfg, meta)

    in_maps = []
    for c in range(NCORES):
        in_maps.append(prep_core_inputs(
            cfg, meta, c, x, W1, a_s1, a_d1, b1, W2, a_s2, a_d2, b2,
            gidx_all[c], drel_all[c]))
    res = run_bass_kernel_spmd(nc, in_maps, list(range(NCORES)))
    outs = [res.results[c]["out"][: cfg.NPC] for c in range(NCORES)]
    return np.concatenate(outs, axis=0)[: cfg.N].astype(np.float32)

